# revision 1
# baseline (speedup 1.0000x reference)
"""NestedParallelBlock TRN2 kernel: host prep + Bass/Tile program + test.

Sharding: 2 cores per batch (4 batches x 2 = 8 cores). Each core owns 512
tokens of its batch (interleaved over the expert-sorted order), computes
expand/attention/contract for them, and redundantly computes K/V for the
other 512 tokens of the batch (attention needs full-batch KV).

Device: expand GEMM (q/mlp feature-major, k/v token-major), LN2 (bn_stats,
rstd=exp(-0.5*ln(var+eps))), k transpose via PE, attention (S^T = K Q^T per
head, exp on ACT with per-k-token rstd folded into the exp scale, A@V with a
fused ones-column giving softmax denominators), contract GEMM (columns
permuted into nested-level blocks), final masked combine as residual delta.
Host: LN1+mask (elementwise on inputs), weight folds/transposes, expert
sort, output assembly (x + scatter(delta)).
"""
import sys, time, os
sys.path.insert(0, '/opt/trn_rl_repo')
from contextlib import ExitStack
import numpy as np
import ml_dtypes

import concourse.bass as bass
import concourse.tile as tile
from concourse import bacc, mybir
from concourse.bass_utils import run_bass_kernel_spmd
from concourse.masks import make_identity

BF16 = ml_dtypes.bfloat16
F32 = mybir.dt.float32
BF = mybir.dt.bfloat16
AF = mybir.ActivationFunctionType
ALU = mybir.AluOpType

D = 1024
H = 16
HD = 64
B, N = 4, 1024
EXP_OUT = 7168   # 3D qkv + 4D mlp
CON_IN = 5120    # D attn + 4D mlp
CON_OUT = 2048
EPS = 1e-5
SCALE = HD ** -0.5
P = 128

# nested output level blocks: (orig_col_start, orig_width)
LEVELS = [(0, 128), (128, 128), (256, 256), (512, 512)]
PERM_W = [2 * w for _, w in LEVELS]            # widths in permuted space
PERM_0 = [0, 256, 512, 1024]                   # block starts in perm space


# ---------------------------------------------------------------- host prep
def build_perm_out():
    idx = []
    for o, w in LEVELS:
        idx.extend(range(o, o + w))
        idx.extend(range(D + o, D + o + w))
    return np.array(idx, np.int64)


def host_prep(inputs, nested=False):
    x = np.asarray(inputs['x'], np.float32)
    mask = np.asarray(inputs['expert_mask']).astype(np.int64)
    probs = np.asarray(inputs['expert_probs'], np.float32)
    we = np.asarray(inputs['expand_weight'], np.float32)
    mlp_bias = np.asarray(inputs['mlp_bias'], np.float32)
    wc = np.asarray(inputs['contract_weight'], np.float32)
    cbias = np.asarray(inputs['contract_bias'], np.float32)
    g1 = np.asarray(inputs['norm1_g'], np.float32); b1 = np.asarray(inputs['norm1_b'], np.float32)
    g2 = np.asarray(inputs['norm2_g'], np.float32); b2 = np.asarray(inputs['norm2_b'], np.float32)
    alpha = np.asarray(inputs['alpha'], np.float32)

    d_e = (128 << mask)                               # [B,N]
    fmask = (np.arange(D)[None, None, :] < d_e[..., None]).astype(np.float32)

    m1 = x.mean(-1, keepdims=True)
    v1 = x.var(-1, keepdims=True)
    xn = ((x - m1) / np.sqrt(v1 + EPS) * g1 + b1) * fmask    # [B,N,D]

    wef = we.copy()
    wef[0:D, :] *= (g2 * SCALE)[:, None]
    qb = mlp_bias[0:D] * g2 * SCALE
    kvb = mlp_bias[D:3 * D].copy()                    # [2048]
    mb = mlp_bias[0:4 * D].copy()
    mb[:D] += mlp_bias[3 * D:4 * D]
    weT = np.ascontiguousarray(wef.T)                 # [1024, 7168]

    wcf = wc.copy()
    cb = cbias + wc[:, 0:D] @ b2
    wcf[:, 0:D] *= g2[None, :]
    perm = build_perm_out()
    wcp = wcf[perm]                                   # [2048, 5120]
    cbp = cb[perm]
    wcT = np.ascontiguousarray(wcp.T)                 # [5120, 2048]
    # pre-tiled for 4KB DMA lines: [10 ks-quads][4 passes][128][4 ks][512]
    wct4 = np.empty((10, 4, 128, 4, 512), np.float32)
    for q4 in range(10):
        for pz in range(4):
            for b4 in range(4):
                rows = slice((q4 * 4 + b4) * 128, (q4 * 4 + b4 + 1) * 128)
                wct4[q4, pz, :, b4, :] = wcT[rows, pz * 512:(pz + 1) * 512]

    cores = []
    for c in range(8):
        b = c // 2
        order = np.argsort(mask[b], kind='stable')
        own = order[c % 2::2]
        rem = order[1 - c % 2::2]
        toks = np.concatenate([own, rem])             # 1024
        xnT = np.ascontiguousarray(xn[b, toks].T).astype(BF16)   # [1024,1024]

        f_own = fmask[b, own]                          # [512, 1024]
        c_own = (alpha[0] * probs[b, own, 0] + 1.0)    # [512]
        w2 = np.empty((512, CON_OUT), np.float32)
        for L, (o, w) in enumerate(LEVELS):
            p0 = PERM_0[L]
            w2[:, p0:p0 + w] = f_own[:, o:o + w]
            w2[:, p0 + w:p0 + 2 * w] = f_own[:, o:o + w] * c_own[:, None]

        cores.append(dict(batch=b, own=own, rem=rem, toks=toks, xnT=xnT, w2=w2,
                          own_lvl=mask[b, own], rem_lvl=mask[b, rem]))

    cfg = dict(ks_start=[0] * 8, kv_nks=[8] * 8, ct_tstart=[0] * 4)
    if nested:
        ks_start = [512] * 8
        kv_nks = [1] * 8
        ct_tstart = [4] * 4
        for ci in cores:
            lvl_own = ci['own_lvl']
            for ks in range(8):
                need = (1 << lvl_own) > ks            # suffix of sorted tokens
                if need.all():
                    first = 0
                elif need.any():
                    first = int(np.argmax(need))
                else:
                    first = 512
                ks_start[ks] = min(ks_start[ks], first)
            lvl_all = np.concatenate([ci['own_lvl'], ci['rem_lvl']])
            for tt in range(8):
                lv = int(lvl_all[tt * 128:(tt + 1) * 128].max())
                kv_nks[tt] = max(kv_nks[tt], 1 << lv)
            for L in range(4):
                need = lvl_own >= L
                first = int(np.argmax(need)) if need.any() else 512
                t0 = (0 if need.all() else first) // 128
                ct_tstart[L] = min(ct_tstart[L], t0)
        cfg = dict(ks_start=[int(v) for v in ks_start],
                   kv_nks=[int(v) for v in kv_nks],
                   ct_tstart=[int(v) for v in ct_tstart])

    sel = np.zeros((H, H * 64), np.float32)
    for h in range(H):
        sel[h, h * 64:(h + 1) * 64] = 1.0
    shared = dict(
        weT=weT.astype(BF16), qb=qb.astype(np.float32), kvb=kvb.astype(BF16)[None, :],
        mb=mb.astype(np.float32), wcT=wct4.astype(BF16), cbp=cbp.astype(BF16)[None, :],
        sel=sel.astype(BF16),
    )
    aux = dict(xn=xn, fmask=fmask, x=x, probs=probs, alpha=alpha, wef=wef,
               mb=mb, qb=qb, kvb=kvb, wcp=wcp, cbp=cbp, mask=mask, g2=g2, b2=b2)
    return shared, cores, cfg, aux


# ---------------------------------------------------------------- builder
def build_program(cfg, debug=()):
    nc = bacc.Bacc('TRN2', target_bir_lowering=False, num_devices=8)

    xnT_d = nc.declare_dram_parameter("xnT", [D, N], BF, isOutput=False)
    weT_d = nc.declare_dram_parameter("weT", [D, EXP_OUT], BF, isOutput=False)
    qb_d = nc.declare_dram_parameter("qb", [D], F32, isOutput=False)
    kvb_d = nc.declare_dram_parameter("kvb", [1, 2 * D], BF, isOutput=False)
    mb_d = nc.declare_dram_parameter("mb", [4 * D], F32, isOutput=False)
    wcT_d = nc.declare_dram_parameter("wcT", [10, 4, P, 4, 512], BF, isOutput=False)
    cbp_d = nc.declare_dram_parameter("cbp", [1, CON_OUT], BF, isOutput=False)
    w2_d = nc.declare_dram_parameter("w2", [512, CON_OUT], F32, isOutput=False)
    sel_d = nc.declare_dram_parameter("sel", [H, H * 64], BF, isOutput=False)
    delta_d = nc.declare_dram_parameter("delta", [512, D], F32, isOutput=True)

    dbg = {}
    def dbg_out(name, shape, dt=F32):
        dbg[name] = nc.declare_dram_parameter(name, shape, dt, isOutput=True)
    if 'qT' in debug: dbg_out('d_qT', [D, 512], BF)
    if 'kT' in debug: dbg_out('d_kT', [D, N], BF)
    if 'vsb' in debug: dbg_out('d_vsb', [N, H * 65], BF)
    if 'rstd' in debug: dbg_out('d_rstd', [N])
    if 'mlpT' in debug: dbg_out('d_mlpT', [4 * D, 512], BF)
    if 'attnT' in debug: dbg_out('d_attnT', [D, 512], BF)

    ks_start = cfg['ks_start']; kv_nks = cfg['kv_nks']; ct_tstart = cfg['ct_tstart']
    q_active_ks = [ks for ks in range(8) if ks_start[ks] < 512]
    q_last_ks = q_active_ks[-1]

    with tile.TileContext(nc) as tc, ExitStack() as ctx:
        consts = ctx.enter_context(tc.tile_pool(name="consts", bufs=1))
        resid = ctx.enter_context(tc.tile_pool(name="resid", bufs=1))
        wstream = ctx.enter_context(tc.tile_pool(name="wstream", bufs=9))
        kvraw_p = ctx.enter_context(tc.tile_pool(name="kvraw", bufs=2))
        stat_p = ctx.enter_context(tc.tile_pool(name="stats", bufs=4))
        small_p = ctx.enter_context(tc.tile_pool(name="small", bufs=2))
        exps_p = ctx.enter_context(tc.tile_pool(name="exps", bufs=3))
        cstream = ctx.enter_context(tc.tile_pool(name="cstream", bufs=2))
        w2s_p = ctx.enter_context(tc.tile_pool(name="w2s", bufs=2))
        dout_p = ctx.enter_context(tc.tile_pool(name="dout", bufs=4))

        ident = consts.tile([P, P], BF)
        make_identity(nc, ident)
        ones_r128 = consts.tile([1, P], BF)
        nc.vector.memset(ones_r128, 1.0)
        ones_r64 = consts.tile([1, 64], BF)
        nc.vector.memset(ones_r64, 1.0)
        eps_t = consts.tile([P, 1], F32)
        nc.vector.memset(eps_t, EPS)
        sel_sb = consts.tile([H, H, 64], BF)
        nc.sync.dma_start(out=sel_sb, in_=sel_d.ap().rearrange("h (g d) -> h g d", d=64))

        # resident inputs
        xnT = resid.tile([P, 8, N], BF)           # [in-feat%128, in-feat//128, token]
        for f in range(8):
            nc.sync.dma_start(out=xnT[:, f, :], in_=xnT_d[f * P:(f + 1) * P, :])
        qb_sb = consts.tile([P, 8], F32)
        nc.sync.dma_start(out=qb_sb, in_=qb_d.ap().rearrange("(f p) -> p f", p=P))
        mb_sb = consts.tile([P, 32], F32)
        nc.sync.dma_start(out=mb_sb, in_=mb_d.ap().rearrange("(f p) -> p f", p=P))
        kvb_sb = consts.tile([1, 2 * D], BF)
        nc.sync.dma_start(out=kvb_sb, in_=kvb_d[:])
        cbp_sb = consts.tile([1, CON_OUT], BF)
        nc.sync.dma_start(out=cbp_sb, in_=cbp_d[:])

        # resident intermediates
        qT = resid.tile([P, 8, 512], BF)
        mlpT = resid.tile([P, 32, 512], BF)
        kT = resid.tile([P, 8, N], BF)
        var_k = resid.tile([P, 8], F32)
        var_v = resid.tile([P, 8], F32)
        rstd_v = resid.tile([P, 8], F32)
        v_sb = resid.tile([P, 8, H, 65], BF)      # [k-token%128, k-tile, head, 64+ones]
        rstd_k = resid.tile([P, 8], F32)
        attnT = resid.tile([P, 8, 512], BF)

        # ---------------- expand phase 1 (q + k/v) psum scope
        exp_scope = ExitStack()
        ps_e = exp_scope.enter_context(tc.tile_pool(name="ps_e", bufs=2, space="PSUM"))
        ps_s = exp_scope.enter_context(tc.tile_pool(name="ps_s", bufs=2, space="PSUM"))

        def expand_group(og, pool):
            # out-feat group of 8 tiles; 8 resident [128,1024] weight tiles
            # (one per K-subtile, 2KB DMA lines) shared by the 8 psum groups
            wg = []
            for ks in q_active_ks:
                w = wstream.tile([P, 8 * P], BF, tag="we_t")
                nc.sync.dma_start(out=w, in_=weT_d[ks * P:(ks + 1) * P,
                                                   og * 8 * P:(og + 1) * 8 * P])
                wg.append((ks, w))
            for oi in range(8):
                of = og * 8 + oi
                ps = pool.tile([P, 512], F32, tag="ps_x")
                for ks, w in wg:
                    s0 = ks_start[ks]
                    nc.tensor.matmul(ps[:, s0:], w[:, oi * P:(oi + 1) * P],
                                     xnT[:, ks, s0:512],
                                     start=(ks == 0), stop=(ks == q_last_ks))
                if of < 8:
                    nc.vector.tensor_scalar_add(out=qT[:, of, :], in0=ps,
                                                scalar1=qb_sb[:, of:of + 1])
                else:
                    f = of - 24
                    nc.vector.tensor_scalar_add(out=mlpT[:, f, :], in0=ps,
                                                scalar1=mb_sb[:, f:f + 1])

        expand_group(0, ps_e)                          # q

        # ---------------- expand: k,v (token-major) + LN2 + k transpose
        # two sequential halves (k then v) so only half the kv weights are
        # resident at a time
        for which in ('k', 'v'):
            coff = D if which == 'k' else 2 * D
            kvw = kvraw_p.tile([P, 8, D], BF, tag="kvw", bufs=1)
            for ks in range(8):
                nc.sync.dma_start(out=kvw[:, ks, :],
                                  in_=weT_d[ks * P:(ks + 1) * P, coff:coff + D])
            for tt in range(8):
                raw = kvraw_p.tile([P, D], F32, tag="raw", bufs=2)
                nks = kv_nks[tt]
                for cf in range(2):               # 512-col tiles
                    ps = ps_e.tile([P, 512], F32, tag="ps_e")
                    nc.tensor.matmul(
                        ps, ones_r128,
                        kvb_sb[0:1, coff - D + cf * 512:coff - D + (cf + 1) * 512],
                        start=True, stop=False)
                    for ks in range(nks):
                        nc.tensor.matmul(ps, xnT[:, ks, tt * P:(tt + 1) * P],
                                         kvw[:, ks, cf * 512:(cf + 1) * 512],
                                         start=False, stop=(ks == nks - 1))
                    nc.vector.tensor_copy(out=raw[:, cf * 512:(cf + 1) * 512], in_=ps)

                stats = stat_p.tile([P, 2, 6], F32, tag="bn_st")
                for i in range(2):
                    nc.vector.bn_stats(out=stats[:, i, :], in_=raw[:, i * 512:(i + 1) * 512])
                mv = stat_p.tile([P, 2], F32, tag="bn_mv")
                nc.vector.bn_aggr(out=mv, in_=stats)
                if which == 'k':
                    nc.vector.tensor_copy(out=var_k[:, tt:tt + 1], in_=mv[:, 1:2])
                    kc = kvraw_p.tile([P, D], BF, tag="kc", bufs=2)
                    nc.vector.tensor_scalar_sub(out=kc, in0=raw, scalar1=mv[:, 0:1])
                    for fg in range(2):           # 4 PE transposes per psum bank
                        tp = ps_s.tile([P, 512], BF, tag="scr")
                        for j in range(4):
                            f = fg * 4 + j
                            nc.tensor.transpose(tp[:, j * P:(j + 1) * P],
                                                kc[:, f * P:(f + 1) * P], ident)
                        nc.vector.tensor_copy(
                            out=kT[:, fg * 4:(fg + 1) * 4, tt * P:(tt + 1) * P], in_=tp)
                else:
                    nc.vector.tensor_copy(out=var_v[:, tt:tt + 1], in_=mv[:, 1:2])
                    nc.vector.tensor_scalar_sub(
                        out=v_sb[:, tt, :, 0:64],
                        in0=raw.rearrange("p (h d) -> p h d", h=H),
                        scalar1=mv[:, 0:1])
                    nc.vector.memset(v_sb[:, tt, :, 64:65], 1.0)
        # batched rstd: one Ln + one Exp per tensor (avoids ACT table thrash)
        for var, rstd in ((var_k, rstd_k), (var_v, rstd_v)):
            lnv = stat_p.tile([P, 8], F32, tag="lnv8")
            nc.scalar.activation(out=lnv, in_=var, func=AF.Ln, bias=eps_t, scale=1.0)
            nc.scalar.activation(out=rstd, in_=lnv, func=AF.Exp, bias=0.0, scale=-0.5)
        for tt in range(8):
            nc.vector.tensor_scalar_mul(out=v_sb[:, tt, :, 0:64],
                                        in0=v_sb[:, tt, :, 0:64],
                                        scalar1=rstd_v[:, tt:tt + 1])

        exp_scope.close()
        # ---------------- attention + mlp expand (attention is ACT-exp-bound;
        # the mlp expand GEMM overlaps it on the PE)
        attnU = resid.tile([P, 8, 512], BF)       # unnormalized head pairs
        den = resid.tile([H, 512], F32)
        att_scope = ExitStack()
        ps_sp = att_scope.enter_context(tc.tile_pool(name="ps_sp", bufs=2, space="PSUM"))
        ps_a = att_scope.enter_context(tc.tile_pool(name="ps_a", bufs=2, space="PSUM"))
        ps_m = att_scope.enter_context(tc.tile_pool(name="ps_m", bufs=2, space="PSUM"))
        for hf in range(8):
            if hf % 2 == 0 and hf // 2 < 4:
                expand_group(3 + hf // 2, ps_m)        # mlp groups between pairs
            h0, h1 = 2 * hf, 2 * hf + 1
            aps0 = ps_a.tile([65, 512], F32, tag="aps")
            aps1 = ps_a.tile([65, 512], F32, tag="aps")
            for kt in range(8):
                sps = ps_sp.tile([P, 1024], F32, tag="sps")
                nc.tensor.matmul(sps[:, 0:512], kT[0:64, hf, kt * P:(kt + 1) * P],
                                 qT[0:64, hf, :], start=True, stop=True)
                nc.tensor.matmul(sps[:, 512:1024], kT[64:128, hf, kt * P:(kt + 1) * P],
                                 qT[64:128, hf, :], start=True, stop=True)
                es = exps_p.tile([P, 1024], BF, tag="es")
                nc.scalar.activation(out=es, in_=sps, func=AF.Exp,
                                     bias=0.0, scale=rstd_k[:, kt:kt + 1])
                nc.tensor.matmul(aps0, v_sb[:, kt, h0, :], es[:, 0:512],
                                 start=(kt == 0), stop=(kt == 7))
                nc.tensor.matmul(aps1, v_sb[:, kt, h1, :], es[:, 512:1024],
                                 start=(kt == 0), stop=(kt == 7))
            nc.vector.tensor_copy(out=attnU[0:64, hf, :], in_=aps0[0:64, :])
            nc.vector.tensor_copy(out=attnU[64:128, hf, :], in_=aps1[0:64, :])
            for h, aps in ((h0, aps0), (h1, aps1)):
                dtmp = small_p.tile([1, 512], F32, tag="dtmp")
                nc.vector.tensor_copy(out=dtmp, in_=aps[64:65, :])
                nc.sync.dma_start(out=den[h:h + 1, :], in_=dtmp)
        att_scope.close()
        # gelu burst: one ACT table switch for all 32 tiles (in place)
        for f in range(32):
            nc.scalar.activation(out=mlpT[:, f, :], in_=mlpT[:, f, :],
                                 func=AF.Gelu, bias=0.0, scale=1.0)
        # normalization tail (overlaps contract: 2 psum banks)
        tail_scope = ExitStack()
        ps_b = tail_scope.enter_context(tc.tile_pool(name="ps_b", bufs=2, space="PSUM"))
        recs = small_p.tile([H, 512], BF, tag="recs", bufs=1)
        with nc.allow_low_precision(reason="softmax denom recip in bf16"):
            nc.vector.reciprocal(out=recs, in_=den)
        for h in range(H):
            hf, hp = h // 2, (h % 2) * 64
            bps = ps_b.tile([64, 512], F32, tag="bps")
            nc.tensor.matmul(bps, sel_sb[:, h, :], recs, start=True, stop=True)
            recb = small_p.tile([P, 512], BF, tag="recb")
            nc.vector.tensor_copy(out=recb[hp:hp + 64, :], in_=bps)
            nc.vector.tensor_mul(out=attnT[hp:hp + 64, hf, :],
                                 in0=attnU[hp:hp + 64, hf, :],
                                 in1=recb[hp:hp + 64, :])

        att_scope.close()
        # ---------------- contract + combine (scoped psum: 5 banks)
        con_scope = ExitStack()
        ps_c = con_scope.enter_context(tc.tile_pool(name="ps_c", bufs=6, space="PSUM"))
        # 4 uniform 512-col passes over the permuted output space; each weight
        # tile [128,512] is loaded once and shared by all live token tiles.
        # P01 = levels 0+1 (pair-combine inside the pass), P2 = level 2,
        # P3a/P3b = the two halves of level 3 (combined at the end).
        passes = [
            dict(p0=0, tstart=min(ct_tstart[0], ct_tstart[1]), kind='p01'),
            dict(p0=512, tstart=ct_tstart[2], kind='p2'),
            dict(p0=1024, tstart=ct_tstart[3], kind='p3a'),
            dict(p0=1536, tstart=ct_tstart[3], kind='p3b'),
        ]
        t1_3a = {}
        for pz in passes:
            p0, t_lo, kind = pz['p0'], pz['tstart'], pz['kind']
            tts = list(range(t_lo, 4))
            cpss = {}
            for t in tts:
                cps = ps_c.tile([P, 512], F32, tag="cps")
                nc.tensor.matmul(cps, ones_r128, cbp_sb[0:1, p0:p0 + 512],
                                 start=True, stop=False)
                cpss[t] = cps
            pz = p0 // 512
            for qi, q4 in enumerate(list(range(2, 10)) + [0, 1]):   # mlp quads first
                wct = cstream.tile([P, 4, 512], BF, tag="wct")
                nc.gpsimd.dma_start(out=wct, in_=wcT_d[q4, pz])
                for b4 in range(4):
                    ks = q4 * 4 + b4
                    if ks < 8:
                        lhsT = attnT[:, ks, t * P:(t + 1) * P]
                    else:
                        lhsT = mlpT[:, ks - 8, t * P:(t + 1) * P]
                    for t in tts:
                        if ks < 8:
                            lhsT = attnT[:, ks, t * P:(t + 1) * P]
                        else:
                            lhsT = mlpT[:, ks - 8, t * P:(t + 1) * P]
                        nc.tensor.matmul(cpss[t], lhsT, wct[:, b4, :], start=False,
                                         stop=(qi == 9 and b4 == 3))
            for t in tts:
                w2t = w2s_p.tile([P, 512], F32, tag="w2t")
                nc.gpsimd.dma_start(out=w2t, in_=w2_d[t * P:(t + 1) * P, p0:p0 + 512])
                t1 = dout_p.tile([P, 512], F32,
                                 tag=("t1keep" if kind == 'p3a' else "t1"),
                                 bufs=(4 if kind == 'p3a' else 2))
                nc.vector.tensor_mul(out=t1, in0=cpss[t], in1=w2t)
                if kind == 'p01':
                    for L in (0, 1):
                        if t < ct_tstart[L]:
                            continue
                        o0, ow = LEVELS[L]
                        q0 = PERM_0[L] - p0
                        dt_ = dout_p.tile([P, 128], F32, tag="dt", bufs=2)
                        nc.vector.tensor_add(out=dt_, in0=t1[:, q0:q0 + ow],
                                             in1=t1[:, q0 + ow:q0 + 2 * ow])
                        nc.sync.dma_start(out=delta_d[t * P:(t + 1) * P, o0:o0 + ow],
                                          in_=dt_)
                elif kind == 'p2':
                    o0, ow = LEVELS[2]
                    dt_ = dout_p.tile([P, 256], F32, tag="dt2", bufs=2)
                    nc.vector.tensor_add(out=dt_, in0=t1[:, 0:ow], in1=t1[:, ow:2 * ow])
                    nc.sync.dma_start(out=delta_d[t * P:(t + 1) * P, o0:o0 + ow], in_=dt_)
                elif kind == 'p3a':
                    t1_3a[t] = t1
                else:
                    o0, ow = LEVELS[3]
                    dt_ = dout_p.tile([P, 512], F32, tag="dt3", bufs=2)
                    nc.vector.tensor_add(out=dt_, in0=t1_3a[t], in1=t1)
                    nc.sync.dma_start(out=delta_d[t * P:(t + 1) * P, o0:o0 + ow], in_=dt_)

        con_scope.close()
        tail_scope.close()
        # ---------------- debug dumps
        if 'd_qT' in dbg:
            for f in range(8):
                nc.sync.dma_start(out=dbg['d_qT'][f * P:(f + 1) * P, :], in_=qT[:, f, :])
        if 'd_mlpT' in dbg:
            for f in range(32):
                nc.sync.dma_start(out=dbg['d_mlpT'][f * P:(f + 1) * P, :], in_=mlpT[:, f, :])
        if 'd_kT' in dbg:
            for f in range(8):
                nc.sync.dma_start(out=dbg['d_kT'][f * P:(f + 1) * P, :], in_=kT[:, f, :])
        if 'd_vsb' in dbg:
            for tt in range(8):
                nc.sync.dma_start(out=dbg['d_vsb'][tt * P:(tt + 1) * P, :],
                                  in_=v_sb[:, tt, :, :])
        if 'd_rstd' in dbg:
            nc.sync.dma_start(out=dbg['d_rstd'].ap().rearrange("(t p) -> p t", p=P),
                              in_=rstd_k)
        if 'd_attnT' in dbg:
            for f in range(8):
                nc.sync.dma_start(out=dbg['d_attnT'][f * P:(f + 1) * P, :], in_=attnT[:, f, :])

    nc.compile()
    return nc


# ---------------------------------------------------------------- entrypoint
_CACHE = {}

def _get_program(cfg_key, cfg):
    if cfg_key not in _CACHE:
        _CACHE[cfg_key] = build_program(cfg)
    return _CACHE[cfg_key]


def kernel(**inputs):
    """Full-input NestedParallelBlock forward on 8 NeuronCores."""
    shared, cores, cfg, aux = host_prep(inputs, nested=True)
    cfg_key = (tuple(cfg['ks_start']), tuple(cfg['kv_nks']), tuple(cfg['ct_tstart']))
    nc = _get_program(cfg_key, cfg)
    in_maps = []
    for c in range(8):
        in_maps.append(dict(xnT=cores[c]['xnT'], weT=shared['weT'], qb=shared['qb'],
                            kvb=shared['kvb'], mb=shared['mb'], wcT=shared['wcT'],
                            cbp=shared['cbp'], w2=cores[c]['w2'], sel=shared['sel']))
    trace = os.environ.get('BASS_KERNEL_TRACE', '') == '1'
    res = run_bass_kernel_spmd(nc, in_maps, list(range(8)), trace=trace)
    LAST_RESULT['exec_time_ns'] = res.exec_time_ns
    x = np.asarray(inputs['x'], np.float32)
    out = x.copy()
    for c in range(8):
        b, own = cores[c]['batch'], cores[c]['own']
        out[b, own, :] += res.results[c]['delta']
    return out


LAST_RESULT = {}


# revision 2
# speedup vs baseline: 1.1281x; 1.1281x over previous
"""NestedParallelBlock TRN2 kernel: host prep + Bass/Tile program + test.

Sharding: 2 cores per batch (4 batches x 2 = 8 cores). Each core owns 512
tokens of its batch (interleaved over the expert-sorted order), computes
expand/attention/contract for them, and redundantly computes K/V for the
other 512 tokens of the batch (attention needs full-batch KV).

Device: expand GEMM (q/mlp feature-major, k/v token-major), LN2 (bn_stats,
rstd=exp(-0.5*ln(var+eps))), k transpose via PE, attention (S^T = K Q^T per
head, exp on ACT with per-k-token rstd folded into the exp scale, A@V with a
fused ones-column giving softmax denominators), contract GEMM (columns
permuted into nested-level blocks), final masked combine as residual delta.
Host: LN1+mask (elementwise on inputs), weight folds/transposes, expert
sort, output assembly (x + scatter(delta)).
"""
import sys, time, os
sys.path.insert(0, '/opt/trn_rl_repo')
from contextlib import ExitStack
import numpy as np
import ml_dtypes

import concourse.bass as bass
import concourse.tile as tile
from concourse import bacc, mybir
from concourse.bass_utils import run_bass_kernel_spmd
from concourse.masks import make_identity

BF16 = ml_dtypes.bfloat16
F32 = mybir.dt.float32
BF = mybir.dt.bfloat16
AF = mybir.ActivationFunctionType
ALU = mybir.AluOpType

D = 1024
H = 16
HD = 64
B, N = 4, 1024
EXP_OUT = 7168   # 3D qkv + 4D mlp
CON_IN = 5120    # D attn + 4D mlp
CON_OUT = 2048
EPS = 1e-5
SCALE = HD ** -0.5
P = 128

# nested output level blocks: (orig_col_start, orig_width)
LEVELS = [(0, 128), (128, 128), (256, 256), (512, 512)]
PERM_W = [2 * w for _, w in LEVELS]            # widths in permuted space
PERM_0 = [0, 256, 512, 1024]                   # block starts in perm space


# ---------------------------------------------------------------- host prep
def build_perm_out():
    idx = []
    for o, w in LEVELS:
        idx.extend(range(o, o + w))
        idx.extend(range(D + o, D + o + w))
    return np.array(idx, np.int64)


def host_prep(inputs, nested=False):
    x = np.asarray(inputs['x'], np.float32)
    mask = np.asarray(inputs['expert_mask']).astype(np.int64)
    probs = np.asarray(inputs['expert_probs'], np.float32)
    we = np.asarray(inputs['expand_weight'], np.float32)
    mlp_bias = np.asarray(inputs['mlp_bias'], np.float32)
    wc = np.asarray(inputs['contract_weight'], np.float32)
    cbias = np.asarray(inputs['contract_bias'], np.float32)
    g1 = np.asarray(inputs['norm1_g'], np.float32); b1 = np.asarray(inputs['norm1_b'], np.float32)
    g2 = np.asarray(inputs['norm2_g'], np.float32); b2 = np.asarray(inputs['norm2_b'], np.float32)
    alpha = np.asarray(inputs['alpha'], np.float32)

    d_e = (128 << mask)                               # [B,N]
    fmask = (np.arange(D)[None, None, :] < d_e[..., None]).astype(np.float32)

    m1 = x.mean(-1, keepdims=True)
    v1 = x.var(-1, keepdims=True)
    xn = ((x - m1) / np.sqrt(v1 + EPS) * g1 + b1) * fmask    # [B,N,D]

    wef = we.copy()
    wef[0:D, :] *= (g2 * SCALE)[:, None]
    qb = mlp_bias[0:D] * g2 * SCALE
    kvb = mlp_bias[D:3 * D].copy()                    # [2048]
    mb = mlp_bias[0:4 * D].copy()
    mb[:D] += mlp_bias[3 * D:4 * D]
    weT = np.ascontiguousarray(wef.T)                 # [1024, 7168]

    wcf = wc.copy()
    cb = cbias + wc[:, 0:D] @ b2
    wcf[:, 0:D] *= g2[None, :]
    perm = build_perm_out()
    wcp = wcf[perm]                                   # [2048, 5120]
    cbp = cb[perm]
    wcT = np.ascontiguousarray(wcp.T)                 # [5120, 2048]
    # pre-tiled for 4KB DMA lines: [10 ks-quads][4 passes][128][4 ks][512]
    wct4 = np.empty((10, 4, 128, 4, 512), np.float32)
    for q4 in range(10):
        for pz in range(4):
            for b4 in range(4):
                rows = slice((q4 * 4 + b4) * 128, (q4 * 4 + b4 + 1) * 128)
                wct4[q4, pz, :, b4, :] = wcT[rows, pz * 512:(pz + 1) * 512]

    cores = []
    for c in range(8):
        b = c // 2
        order = np.argsort(mask[b], kind='stable')
        own = order[c % 2::2]
        rem = order[1 - c % 2::2]
        toks = np.concatenate([own, rem])             # 1024
        xnT = np.ascontiguousarray(xn[b, toks].T).astype(BF16)   # [1024,1024]

        f_own = fmask[b, own]                          # [512, 1024]
        c_own = (alpha[0] * probs[b, own, 0] + 1.0)    # [512]
        w2 = np.empty((512, CON_OUT), np.float32)
        for L, (o, w) in enumerate(LEVELS):
            p0 = PERM_0[L]
            w2[:, p0:p0 + w] = f_own[:, o:o + w]
            w2[:, p0 + w:p0 + 2 * w] = f_own[:, o:o + w] * c_own[:, None]

        cores.append(dict(batch=b, own=own, rem=rem, toks=toks, xnT=xnT, w2=w2,
                          own_lvl=mask[b, own], rem_lvl=mask[b, rem]))

    cfg = dict(ks_start=[0] * 8, kv_nks=[8] * 8, ct_tstart=[0] * 4)
    if nested:
        ks_start = [512] * 8
        kv_nks = [1] * 8
        ct_tstart = [4] * 4
        for ci in cores:
            lvl_own = ci['own_lvl']
            for ks in range(8):
                need = (1 << lvl_own) > ks            # suffix of sorted tokens
                if need.all():
                    first = 0
                elif need.any():
                    first = int(np.argmax(need))
                else:
                    first = 512
                ks_start[ks] = min(ks_start[ks], first)
            lvl_all = np.concatenate([ci['own_lvl'], ci['rem_lvl']])
            for tt in range(8):
                lv = int(lvl_all[tt * 128:(tt + 1) * 128].max())
                kv_nks[tt] = max(kv_nks[tt], 1 << lv)
            for L in range(4):
                need = lvl_own >= L
                first = int(np.argmax(need)) if need.any() else 512
                t0 = (0 if need.all() else first) // 128
                ct_tstart[L] = min(ct_tstart[L], t0)
        cfg = dict(ks_start=[int(v) for v in ks_start],
                   kv_nks=[int(v) for v in kv_nks],
                   ct_tstart=[int(v) for v in ct_tstart])

    sel = np.zeros((H, H * 64), np.float32)
    for h in range(H):
        sel[h, h * 64:(h + 1) * 64] = 1.0
    shared = dict(
        weT=weT.astype(BF16), qb=qb.astype(np.float32), kvb=kvb.astype(BF16)[None, :],
        mb=mb.astype(np.float32), wcT=wct4.astype(BF16), cbp=cbp.astype(BF16)[None, :],
        sel=sel.astype(BF16),
    )
    aux = dict(xn=xn, fmask=fmask, x=x, probs=probs, alpha=alpha, wef=wef,
               mb=mb, qb=qb, kvb=kvb, wcp=wcp, cbp=cbp, mask=mask, g2=g2, b2=b2)
    return shared, cores, cfg, aux


# ---------------------------------------------------------------- builder
def build_program(cfg, debug=()):
    nc = bacc.Bacc('TRN2', target_bir_lowering=False, num_devices=8)

    xnT_d = nc.declare_dram_parameter("xnT", [D, N], BF, isOutput=False)
    weT_d = nc.declare_dram_parameter("weT", [D, EXP_OUT], BF, isOutput=False)
    qb_d = nc.declare_dram_parameter("qb", [D], F32, isOutput=False)
    kvb_d = nc.declare_dram_parameter("kvb", [1, 2 * D], BF, isOutput=False)
    mb_d = nc.declare_dram_parameter("mb", [4 * D], F32, isOutput=False)
    wcT_d = nc.declare_dram_parameter("wcT", [10, 4, P, 4, 512], BF, isOutput=False)
    cbp_d = nc.declare_dram_parameter("cbp", [1, CON_OUT], BF, isOutput=False)
    w2_d = nc.declare_dram_parameter("w2", [512, CON_OUT], F32, isOutput=False)
    sel_d = nc.declare_dram_parameter("sel", [H, H * 64], BF, isOutput=False)
    delta_d = nc.declare_dram_parameter("delta", [512, D], F32, isOutput=True)

    dbg = {}
    def dbg_out(name, shape, dt=F32):
        dbg[name] = nc.declare_dram_parameter(name, shape, dt, isOutput=True)
    if 'qT' in debug: dbg_out('d_qT', [D, 512], BF)
    if 'kT' in debug: dbg_out('d_kT', [D, N], BF)
    if 'vsb' in debug: dbg_out('d_vsb', [N, H * 65], BF)
    if 'rstd' in debug: dbg_out('d_rstd', [N])
    if 'mlpT' in debug: dbg_out('d_mlpT', [4 * D, 512], BF)
    if 'attnT' in debug: dbg_out('d_attnT', [D, 512], BF)

    ks_start = cfg['ks_start']; kv_nks = cfg['kv_nks']; ct_tstart = cfg['ct_tstart']
    q_active_ks = [ks for ks in range(8) if ks_start[ks] < 512]
    q_last_ks = q_active_ks[-1]

    with tile.TileContext(nc) as tc, ExitStack() as ctx:
        consts = ctx.enter_context(tc.tile_pool(name="consts", bufs=1))
        resid = ctx.enter_context(tc.tile_pool(name="resid", bufs=1))
        wstream = ctx.enter_context(tc.tile_pool(name="wstream", bufs=9))
        kvraw_p = ctx.enter_context(tc.tile_pool(name="kvraw", bufs=2))
        stat_p = ctx.enter_context(tc.tile_pool(name="stats", bufs=4))
        small_p = ctx.enter_context(tc.tile_pool(name="small", bufs=2))
        exps_p = ctx.enter_context(tc.tile_pool(name="exps", bufs=3))
        cstream = ctx.enter_context(tc.tile_pool(name="cstream", bufs=2))
        w2s_p = ctx.enter_context(tc.tile_pool(name="w2s", bufs=2))
        dout_p = ctx.enter_context(tc.tile_pool(name="dout", bufs=4))

        ident = consts.tile([P, P], BF)
        make_identity(nc, ident)
        ones_r128 = consts.tile([1, P], BF)
        nc.vector.memset(ones_r128, 1.0)
        ones_r64 = consts.tile([1, 64], BF)
        nc.vector.memset(ones_r64, 1.0)
        eps_t = consts.tile([P, 1], F32)
        nc.vector.memset(eps_t, EPS)
        sel_sb = consts.tile([H, H, 64], BF)
        nc.sync.dma_start(out=sel_sb, in_=sel_d.ap().rearrange("h (g d) -> h g d", d=64))

        # resident inputs
        xnT = resid.tile([P, 8, N], BF)           # [in-feat%128, in-feat//128, token]
        for f in range(8):
            nc.sync.dma_start(out=xnT[:, f, :], in_=xnT_d[f * P:(f + 1) * P, :])
        qb_sb = consts.tile([P, 8], F32)
        nc.sync.dma_start(out=qb_sb, in_=qb_d.ap().rearrange("(f p) -> p f", p=P))
        mb_sb = consts.tile([P, 32], F32)
        nc.sync.dma_start(out=mb_sb, in_=mb_d.ap().rearrange("(f p) -> p f", p=P))
        kvb_sb = consts.tile([1, 2 * D], BF)
        nc.sync.dma_start(out=kvb_sb, in_=kvb_d[:])
        cbp_sb = consts.tile([1, CON_OUT], BF)
        nc.sync.dma_start(out=cbp_sb, in_=cbp_d[:])

        # resident intermediates
        qT = resid.tile([P, 8, 512], BF)
        mlpT = resid.tile([P, 32, 512], BF)
        kT = resid.tile([P, 8, N], BF)
        var_k = resid.tile([P, 8], F32)
        var_v = resid.tile([P, 8], F32)
        rstd_v = resid.tile([P, 8], F32)
        v_sb = resid.tile([P, 8, H, 65], BF)      # [k-token%128, k-tile, head, 64+ones]
        rstd_k = resid.tile([P, 8], F32)
        attnT = resid.tile([P, 8, 512], BF)

        # ---------------- expand phase 1 (q + k/v) psum scope
        exp_scope = ExitStack()
        ps_e = exp_scope.enter_context(tc.tile_pool(name="ps_e", bufs=2, space="PSUM"))
        ps_s = exp_scope.enter_context(tc.tile_pool(name="ps_s", bufs=2, space="PSUM"))

        def expand_group(og, pool):
            # out-feat group of 8 tiles; 8 resident [128,1024] weight tiles
            # (one per K-subtile, 2KB DMA lines) shared by the 8 psum groups
            wg = []
            for ks in q_active_ks:
                w = wstream.tile([P, 8 * P], BF, tag="we_t")
                nc.sync.dma_start(out=w, in_=weT_d[ks * P:(ks + 1) * P,
                                                   og * 8 * P:(og + 1) * 8 * P])
                wg.append((ks, w))
            for oi in range(8):
                of = og * 8 + oi
                ps = pool.tile([P, 512], F32, tag="ps_x")
                for ks, w in wg:
                    s0 = ks_start[ks]
                    nc.tensor.matmul(ps[:, s0:], w[:, oi * P:(oi + 1) * P],
                                     xnT[:, ks, s0:512],
                                     start=(ks == 0), stop=(ks == q_last_ks))
                if of < 8:
                    nc.vector.tensor_scalar_add(out=qT[:, of, :], in0=ps,
                                                scalar1=qb_sb[:, of:of + 1])
                else:
                    f = of - 24
                    nc.vector.tensor_scalar_add(out=mlpT[:, f, :], in0=ps,
                                                scalar1=mb_sb[:, f:f + 1])

        expand_group(0, ps_e)                          # q

        # ---------------- expand: k,v (token-major) + LN2 + k transpose
        # two sequential halves (k then v) so only half the kv weights are
        # resident at a time
        for which in ('k', 'v'):
            coff = D if which == 'k' else 2 * D
            kvw = kvraw_p.tile([P, 8, D], BF, tag="kvw", bufs=1)
            for ks in range(8):
                nc.sync.dma_start(out=kvw[:, ks, :],
                                  in_=weT_d[ks * P:(ks + 1) * P, coff:coff + D])
            for tt in range(8):
                raw = kvraw_p.tile([P, D], F32, tag="raw", bufs=2)
                nks = kv_nks[tt]
                for cf in range(2):               # 512-col tiles
                    ps = ps_e.tile([P, 512], F32, tag="ps_e")
                    nc.tensor.matmul(
                        ps, ones_r128,
                        kvb_sb[0:1, coff - D + cf * 512:coff - D + (cf + 1) * 512],
                        start=True, stop=False)
                    for ks in range(nks):
                        nc.tensor.matmul(ps, xnT[:, ks, tt * P:(tt + 1) * P],
                                         kvw[:, ks, cf * 512:(cf + 1) * 512],
                                         start=False, stop=(ks == nks - 1))
                    nc.vector.tensor_copy(out=raw[:, cf * 512:(cf + 1) * 512], in_=ps)

                stats = stat_p.tile([P, 2, 6], F32, tag="bn_st")
                for i in range(2):
                    nc.vector.bn_stats(out=stats[:, i, :], in_=raw[:, i * 512:(i + 1) * 512])
                mv = stat_p.tile([P, 2], F32, tag="bn_mv")
                nc.vector.bn_aggr(out=mv, in_=stats)
                if which == 'k':
                    nc.vector.tensor_copy(out=var_k[:, tt:tt + 1], in_=mv[:, 1:2])
                    kc = kvraw_p.tile([P, D], BF, tag="kc", bufs=2)
                    nc.vector.tensor_scalar_sub(out=kc, in0=raw, scalar1=mv[:, 0:1])
                    for fg in range(2):           # 4 PE transposes per psum bank
                        tp = ps_s.tile([P, 512], BF, tag="scr")
                        for j in range(4):
                            f = fg * 4 + j
                            nc.tensor.transpose(tp[:, j * P:(j + 1) * P],
                                                kc[:, f * P:(f + 1) * P], ident)
                        nc.vector.tensor_copy(
                            out=kT[:, fg * 4:(fg + 1) * 4, tt * P:(tt + 1) * P], in_=tp)
                else:
                    nc.vector.tensor_copy(out=var_v[:, tt:tt + 1], in_=mv[:, 1:2])
                    nc.vector.tensor_scalar_sub(
                        out=v_sb[:, tt, :, 0:64],
                        in0=raw.rearrange("p (h d) -> p h d", h=H),
                        scalar1=mv[:, 0:1])
                    nc.vector.memset(v_sb[:, tt, :, 64:65], 1.0)
        # batched rstd: one Ln + one Exp per tensor (avoids ACT table thrash)
        for var, rstd in ((var_k, rstd_k), (var_v, rstd_v)):
            lnv = stat_p.tile([P, 8], F32, tag="lnv8")
            nc.scalar.activation(out=lnv, in_=var, func=AF.Ln, bias=eps_t, scale=1.0)
            nc.scalar.activation(out=rstd, in_=lnv, func=AF.Exp, bias=0.0, scale=-0.5)
        for tt in range(8):
            nc.vector.tensor_scalar_mul(out=v_sb[:, tt, :, 0:64],
                                        in0=v_sb[:, tt, :, 0:64],
                                        scalar1=rstd_v[:, tt:tt + 1])

        exp_scope.close()
        # ---------------- attention + mlp expand (attention is ACT-exp-bound;
        # the mlp expand GEMM overlaps it on the PE)
        attnU = resid.tile([P, 8, 512], BF)       # unnormalized head pairs
        den = resid.tile([H, 512], F32)
        att_scope = ExitStack()
        ps_sp = att_scope.enter_context(tc.tile_pool(name="ps_sp", bufs=2, space="PSUM"))
        ps_a = att_scope.enter_context(tc.tile_pool(name="ps_a", bufs=2, space="PSUM"))
        ps_m = att_scope.enter_context(tc.tile_pool(name="ps_m", bufs=2, space="PSUM"))
        for hf in range(8):
            if hf % 2 == 0 and hf // 2 < 4:
                expand_group(3 + hf // 2, ps_m)        # mlp groups between pairs
            h0, h1 = 2 * hf, 2 * hf + 1
            aps0 = ps_a.tile([65, 512], F32, tag="aps")
            aps1 = ps_a.tile([65, 512], F32, tag="aps")
            for kt in range(8):
                sps = ps_sp.tile([P, 1024], F32, tag="sps")
                nc.tensor.matmul(sps[:, 0:512], kT[0:64, hf, kt * P:(kt + 1) * P],
                                 qT[0:64, hf, :], start=True, stop=True)
                nc.tensor.matmul(sps[:, 512:1024], kT[64:128, hf, kt * P:(kt + 1) * P],
                                 qT[64:128, hf, :], start=True, stop=True)
                es = exps_p.tile([P, 1024], BF, tag="es")
                nc.scalar.activation(out=es, in_=sps, func=AF.Exp,
                                     bias=0.0, scale=rstd_k[:, kt:kt + 1])
                nc.tensor.matmul(aps0, v_sb[:, kt, h0, :], es[:, 0:512],
                                 start=(kt == 0), stop=(kt == 7))
                nc.tensor.matmul(aps1, v_sb[:, kt, h1, :], es[:, 512:1024],
                                 start=(kt == 0), stop=(kt == 7))
            nc.vector.tensor_copy(out=attnU[0:64, hf, :], in_=aps0[0:64, :])
            nc.vector.tensor_copy(out=attnU[64:128, hf, :], in_=aps1[0:64, :])
            for h, aps in ((h0, aps0), (h1, aps1)):
                dtmp = small_p.tile([1, 512], F32, tag="dtmp")
                nc.vector.tensor_copy(out=dtmp, in_=aps[64:65, :])
                nc.sync.dma_start(out=den[h:h + 1, :], in_=dtmp)
        att_scope.close()
        # gelu burst: two giant in-place ops (bounds ACT table switches at 2)
        for hb in range(2):
            nc.scalar.activation(out=mlpT[:, hb * 16:(hb + 1) * 16, :],
                                 in_=mlpT[:, hb * 16:(hb + 1) * 16, :],
                                 func=AF.Gelu, bias=0.0, scale=1.0)
        # normalization tail (overlaps contract: 2 psum banks)
        tail_scope = ExitStack()
        ps_b = tail_scope.enter_context(tc.tile_pool(name="ps_b", bufs=2, space="PSUM"))
        recs = small_p.tile([H, 512], BF, tag="recs", bufs=1)
        with nc.allow_low_precision(reason="softmax denom recip in bf16"):
            nc.vector.reciprocal(out=recs, in_=den)
        for h in range(H):
            hf, hp = h // 2, (h % 2) * 64
            bps = ps_b.tile([64, 512], F32, tag="bps")
            nc.tensor.matmul(bps, sel_sb[:, h, :], recs, start=True, stop=True)
            recb = small_p.tile([P, 512], BF, tag="recb")
            nc.vector.tensor_copy(out=recb[hp:hp + 64, :], in_=bps)
            nc.vector.tensor_mul(out=attnT[hp:hp + 64, hf, :],
                                 in0=attnU[hp:hp + 64, hf, :],
                                 in1=recb[hp:hp + 64, :])

        att_scope.close()
        # ---------------- contract + combine (scoped psum: 5 banks)
        con_scope = ExitStack()
        ps_c = con_scope.enter_context(tc.tile_pool(name="ps_c", bufs=6, space="PSUM"))
        # 4 uniform 512-col passes over the permuted output space; each weight
        # tile [128,512] is loaded once and shared by all live token tiles.
        # P01 = levels 0+1 (pair-combine inside the pass), P2 = level 2,
        # P3a/P3b = the two halves of level 3 (combined at the end).
        passes = [
            dict(p0=0, tstart=min(ct_tstart[0], ct_tstart[1]), kind='p01'),
            dict(p0=512, tstart=ct_tstart[2], kind='p2'),
            dict(p0=1024, tstart=ct_tstart[3], kind='p3a'),
            dict(p0=1536, tstart=ct_tstart[3], kind='p3b'),
        ]
        t1_3a = {}
        for pz in passes:
            p0, t_lo, kind = pz['p0'], pz['tstart'], pz['kind']
            tts = list(range(t_lo, 4))
            cpss = {}
            for t in tts:
                cps = ps_c.tile([P, 512], F32, tag="cps")
                nc.tensor.matmul(cps, ones_r128, cbp_sb[0:1, p0:p0 + 512],
                                 start=True, stop=False)
                cpss[t] = cps
            pz = p0 // 512
            for qi, q4 in enumerate(list(range(2, 10)) + [0, 1]):   # mlp quads first
                wct = cstream.tile([P, 4, 512], BF, tag="wct")
                nc.gpsimd.dma_start(out=wct, in_=wcT_d[q4, pz])
                for b4 in range(4):
                    ks = q4 * 4 + b4
                    if ks < 8:
                        lhsT = attnT[:, ks, t * P:(t + 1) * P]
                    else:
                        lhsT = mlpT[:, ks - 8, t * P:(t + 1) * P]
                    for t in tts:
                        if ks < 8:
                            lhsT = attnT[:, ks, t * P:(t + 1) * P]
                        else:
                            lhsT = mlpT[:, ks - 8, t * P:(t + 1) * P]
                        nc.tensor.matmul(cpss[t], lhsT, wct[:, b4, :], start=False,
                                         stop=(qi == 9 and b4 == 3))
            for t in tts:
                w2t = w2s_p.tile([P, 512], F32, tag="w2t")
                nc.gpsimd.dma_start(out=w2t, in_=w2_d[t * P:(t + 1) * P, p0:p0 + 512])
                t1 = dout_p.tile([P, 512], F32,
                                 tag=("t1keep" if kind == 'p3a' else "t1"),
                                 bufs=(4 if kind == 'p3a' else 2))
                nc.vector.tensor_mul(out=t1, in0=cpss[t], in1=w2t)
                if kind == 'p01':
                    for L in (0, 1):
                        if t < ct_tstart[L]:
                            continue
                        o0, ow = LEVELS[L]
                        q0 = PERM_0[L] - p0
                        dt_ = dout_p.tile([P, 128], F32, tag="dt", bufs=2)
                        nc.vector.tensor_add(out=dt_, in0=t1[:, q0:q0 + ow],
                                             in1=t1[:, q0 + ow:q0 + 2 * ow])
                        nc.sync.dma_start(out=delta_d[t * P:(t + 1) * P, o0:o0 + ow],
                                          in_=dt_)
                elif kind == 'p2':
                    o0, ow = LEVELS[2]
                    dt_ = dout_p.tile([P, 256], F32, tag="dt2", bufs=2)
                    nc.vector.tensor_add(out=dt_, in0=t1[:, 0:ow], in1=t1[:, ow:2 * ow])
                    nc.sync.dma_start(out=delta_d[t * P:(t + 1) * P, o0:o0 + ow], in_=dt_)
                elif kind == 'p3a':
                    t1_3a[t] = t1
                else:
                    o0, ow = LEVELS[3]
                    dt_ = dout_p.tile([P, 512], F32, tag="dt3", bufs=2)
                    nc.vector.tensor_add(out=dt_, in0=t1_3a[t], in1=t1)
                    nc.sync.dma_start(out=delta_d[t * P:(t + 1) * P, o0:o0 + ow], in_=dt_)

        con_scope.close()
        tail_scope.close()
        # ---------------- debug dumps
        if 'd_qT' in dbg:
            for f in range(8):
                nc.sync.dma_start(out=dbg['d_qT'][f * P:(f + 1) * P, :], in_=qT[:, f, :])
        if 'd_mlpT' in dbg:
            for f in range(32):
                nc.sync.dma_start(out=dbg['d_mlpT'][f * P:(f + 1) * P, :], in_=mlpT[:, f, :])
        if 'd_kT' in dbg:
            for f in range(8):
                nc.sync.dma_start(out=dbg['d_kT'][f * P:(f + 1) * P, :], in_=kT[:, f, :])
        if 'd_vsb' in dbg:
            for tt in range(8):
                nc.sync.dma_start(out=dbg['d_vsb'][tt * P:(tt + 1) * P, :],
                                  in_=v_sb[:, tt, :, :])
        if 'd_rstd' in dbg:
            nc.sync.dma_start(out=dbg['d_rstd'].ap().rearrange("(t p) -> p t", p=P),
                              in_=rstd_k)
        if 'd_attnT' in dbg:
            for f in range(8):
                nc.sync.dma_start(out=dbg['d_attnT'][f * P:(f + 1) * P, :], in_=attnT[:, f, :])

    nc.compile()
    return nc


# ---------------------------------------------------------------- entrypoint
_CACHE = {}

def _get_program(cfg_key, cfg):
    if cfg_key not in _CACHE:
        _CACHE[cfg_key] = build_program(cfg)
    return _CACHE[cfg_key]


def kernel(**inputs):
    """Full-input NestedParallelBlock forward on 8 NeuronCores."""
    shared, cores, cfg, aux = host_prep(inputs, nested=True)
    cfg_key = (tuple(cfg['ks_start']), tuple(cfg['kv_nks']), tuple(cfg['ct_tstart']))
    nc = _get_program(cfg_key, cfg)
    in_maps = []
    for c in range(8):
        in_maps.append(dict(xnT=cores[c]['xnT'], weT=shared['weT'], qb=shared['qb'],
                            kvb=shared['kvb'], mb=shared['mb'], wcT=shared['wcT'],
                            cbp=shared['cbp'], w2=cores[c]['w2'], sel=shared['sel']))
    trace = os.environ.get('BASS_KERNEL_TRACE', '') == '1'
    res = run_bass_kernel_spmd(nc, in_maps, list(range(8)), trace=trace)
    LAST_RESULT['exec_time_ns'] = res.exec_time_ns
    x = np.asarray(inputs['x'], np.float32)
    out = x.copy()
    for c in range(8):
        b, own = cores[c]['batch'], cores[c]['own']
        out[b, own, :] += res.results[c]['delta']
    return out


LAST_RESULT = {}


# revision 4
# speedup vs baseline: 1.1571x; 1.0257x over previous
"""NestedParallelBlock TRN2 kernel: host prep + Bass/Tile program + test.

Sharding: 2 cores per batch (4 batches x 2 = 8 cores). Each core owns 512
tokens of its batch (interleaved over the expert-sorted order), computes
expand/attention/contract for them, and redundantly computes K/V for the
other 512 tokens of the batch (attention needs full-batch KV).

Device: expand GEMM (q/mlp feature-major, k/v token-major), LN2 (bn_stats,
rstd=exp(-0.5*ln(var+eps))), k transpose via PE, attention (S^T = K Q^T per
head, exp on ACT with per-k-token rstd folded into the exp scale, A@V with a
fused ones-column giving softmax denominators), contract GEMM (columns
permuted into nested-level blocks), final masked combine as residual delta.
Host: LN1+mask (elementwise on inputs), weight folds/transposes, expert
sort, output assembly (x + scatter(delta)).
"""
import sys, time, os
sys.path.insert(0, '/opt/trn_rl_repo')
from contextlib import ExitStack
import numpy as np
import ml_dtypes

import concourse.bass as bass
import concourse.tile as tile
from concourse import bacc, mybir
from concourse.bass_utils import run_bass_kernel_spmd
from concourse.masks import make_identity

BF16 = ml_dtypes.bfloat16
F32 = mybir.dt.float32
BF = mybir.dt.bfloat16
AF = mybir.ActivationFunctionType
ALU = mybir.AluOpType

D = 1024
H = 16
HD = 64
B, N = 4, 1024
EXP_OUT = 7168   # 3D qkv + 4D mlp
CON_IN = 5120    # D attn + 4D mlp
CON_OUT = 2048
EPS = 1e-5
SCALE = HD ** -0.5
P = 128

# nested output level blocks: (orig_col_start, orig_width)
LEVELS = [(0, 128), (128, 128), (256, 256), (512, 512)]
PERM_W = [2 * w for _, w in LEVELS]            # widths in permuted space
PERM_0 = [0, 256, 512, 1024]                   # block starts in perm space


# ---------------------------------------------------------------- host prep
def build_perm_out():
    idx = []
    for o, w in LEVELS:
        idx.extend(range(o, o + w))
        idx.extend(range(D + o, D + o + w))
    return np.array(idx, np.int64)


def host_prep(inputs, nested=False):
    x = np.asarray(inputs['x'], np.float32)
    mask = np.asarray(inputs['expert_mask']).astype(np.int64)
    probs = np.asarray(inputs['expert_probs'], np.float32)
    we = np.asarray(inputs['expand_weight'], np.float32)
    mlp_bias = np.asarray(inputs['mlp_bias'], np.float32)
    wc = np.asarray(inputs['contract_weight'], np.float32)
    cbias = np.asarray(inputs['contract_bias'], np.float32)
    g1 = np.asarray(inputs['norm1_g'], np.float32); b1 = np.asarray(inputs['norm1_b'], np.float32)
    g2 = np.asarray(inputs['norm2_g'], np.float32); b2 = np.asarray(inputs['norm2_b'], np.float32)
    alpha = np.asarray(inputs['alpha'], np.float32)

    d_e = (128 << mask)                               # [B,N]
    fmask = (np.arange(D)[None, None, :] < d_e[..., None]).astype(np.float32)

    m1 = x.mean(-1, keepdims=True)
    v1 = x.var(-1, keepdims=True)
    xn = ((x - m1) / np.sqrt(v1 + EPS) * g1 + b1) * fmask    # [B,N,D]

    wef = we.copy()
    wef[0:D, :] *= (g2 * SCALE)[:, None]
    qb = mlp_bias[0:D] * g2 * SCALE
    kvb = mlp_bias[D:3 * D].copy()                    # [2048]
    mb = mlp_bias[0:4 * D].copy()
    mb[:D] += mlp_bias[3 * D:4 * D]
    weT = np.ascontiguousarray(wef.T)                 # [1024, 7168]

    wcf = wc.copy()
    cb = cbias + wc[:, 0:D] @ b2
    wcf[:, 0:D] *= g2[None, :]
    perm = build_perm_out()
    wcp = wcf[perm]                                   # [2048, 5120]
    cbp = cb[perm]
    wcT = np.ascontiguousarray(wcp.T)                 # [5120, 2048]
    # pre-tiled for 4KB DMA lines: [10 ks-quads][4 passes][128][4 ks][512]
    wct4 = np.empty((10, 4, 128, 4, 512), np.float32)
    for q4 in range(10):
        for pz in range(4):
            for b4 in range(4):
                rows = slice((q4 * 4 + b4) * 128, (q4 * 4 + b4 + 1) * 128)
                wct4[q4, pz, :, b4, :] = wcT[rows, pz * 512:(pz + 1) * 512]

    cores = []
    for c in range(8):
        b = c // 2
        order = np.argsort(mask[b], kind='stable')
        own = order[c % 2::2]
        rem = order[1 - c % 2::2]
        toks = np.concatenate([own, rem])             # 1024
        xnT = np.ascontiguousarray(xn[b, toks].T).astype(BF16)   # [1024,1024]

        f_own = fmask[b, own]                          # [512, 1024]
        c_own = (alpha[0] * probs[b, own, 0] + 1.0)    # [512]
        w2 = np.empty((512, CON_OUT), np.float32)
        for L, (o, w) in enumerate(LEVELS):
            p0 = PERM_0[L]
            w2[:, p0:p0 + w] = f_own[:, o:o + w]
            w2[:, p0 + w:p0 + 2 * w] = f_own[:, o:o + w] * c_own[:, None]

        cores.append(dict(batch=b, own=own, rem=rem, toks=toks, xnT=xnT, w2=w2,
                          own_lvl=mask[b, own], rem_lvl=mask[b, rem]))

    cfg = dict(ks_start=[0] * 8, kv_nks=[8] * 8, ct_tstart=[0] * 4)
    if nested:
        ks_start = [512] * 8
        kv_nks = [1] * 8
        ct_tstart = [4] * 4
        for ci in cores:
            lvl_own = ci['own_lvl']
            for ks in range(8):
                need = (1 << lvl_own) > ks            # suffix of sorted tokens
                if need.all():
                    first = 0
                elif need.any():
                    first = int(np.argmax(need))
                else:
                    first = 512
                ks_start[ks] = min(ks_start[ks], first)
            lvl_all = np.concatenate([ci['own_lvl'], ci['rem_lvl']])
            for tt in range(8):
                lv = int(lvl_all[tt * 128:(tt + 1) * 128].max())
                kv_nks[tt] = max(kv_nks[tt], 1 << lv)
            for L in range(4):
                need = lvl_own >= L
                first = int(np.argmax(need)) if need.any() else 512
                t0 = (0 if need.all() else first) // 128
                ct_tstart[L] = min(ct_tstart[L], t0)
        cfg = dict(ks_start=[int(v) for v in ks_start],
                   kv_nks=[int(v) for v in kv_nks],
                   ct_tstart=[int(v) for v in ct_tstart])

    sel = np.zeros((H, H * 64), np.float32)
    for h in range(H):
        sel[h, h * 64:(h + 1) * 64] = 1.0
    shared = dict(
        weT=weT.astype(BF16), qb=qb.astype(np.float32), kvb=kvb.astype(BF16)[None, :],
        mb=mb.astype(np.float32), wcT=wct4.astype(BF16), cbp=cbp.astype(BF16)[None, :],
        sel=sel.astype(BF16),
    )
    aux = dict(xn=xn, fmask=fmask, x=x, probs=probs, alpha=alpha, wef=wef,
               mb=mb, qb=qb, kvb=kvb, wcp=wcp, cbp=cbp, mask=mask, g2=g2, b2=b2)
    return shared, cores, cfg, aux


# ---------------------------------------------------------------- builder
def build_program(cfg, debug=()):
    nc = bacc.Bacc('TRN2', target_bir_lowering=False, num_devices=8)

    xnT_d = nc.declare_dram_parameter("xnT", [D, N], BF, isOutput=False)
    weT_d = nc.declare_dram_parameter("weT", [D, EXP_OUT], BF, isOutput=False)
    qb_d = nc.declare_dram_parameter("qb", [D], F32, isOutput=False)
    kvb_d = nc.declare_dram_parameter("kvb", [1, 2 * D], BF, isOutput=False)
    mb_d = nc.declare_dram_parameter("mb", [4 * D], F32, isOutput=False)
    wcT_d = nc.declare_dram_parameter("wcT", [10, 4, P, 4, 512], BF, isOutput=False)
    cbp_d = nc.declare_dram_parameter("cbp", [1, CON_OUT], BF, isOutput=False)
    w2_d = nc.declare_dram_parameter("w2", [512, CON_OUT], F32, isOutput=False)
    sel_d = nc.declare_dram_parameter("sel", [H, H * 64], BF, isOutput=False)
    delta_d = nc.declare_dram_parameter("delta", [512, D], F32, isOutput=True)

    dbg = {}
    def dbg_out(name, shape, dt=F32):
        dbg[name] = nc.declare_dram_parameter(name, shape, dt, isOutput=True)
    if 'qT' in debug: dbg_out('d_qT', [D, 512], BF)
    if 'kT' in debug: dbg_out('d_kT', [D, N], BF)
    if 'vsb' in debug: dbg_out('d_vsb', [N, H * 65], BF)
    if 'rstd' in debug: dbg_out('d_rstd', [N])
    if 'mlpT' in debug: dbg_out('d_mlpT', [4 * D, 512], BF)
    if 'attnT' in debug: dbg_out('d_attnT', [D, 512], BF)

    ks_start = cfg['ks_start']; kv_nks = cfg['kv_nks']; ct_tstart = cfg['ct_tstart']
    q_active_ks = [ks for ks in range(8) if ks_start[ks] < 512]
    q_last_ks = q_active_ks[-1]

    with tile.TileContext(nc) as tc, ExitStack() as ctx:
        consts = ctx.enter_context(tc.tile_pool(name="consts", bufs=1))
        resid = ctx.enter_context(tc.tile_pool(name="resid", bufs=1))
        wstream = ctx.enter_context(tc.tile_pool(name="wstream", bufs=9))
        kvraw_p = ctx.enter_context(tc.tile_pool(name="kvraw", bufs=2))
        stat_p = ctx.enter_context(tc.tile_pool(name="stats", bufs=4))
        small_p = ctx.enter_context(tc.tile_pool(name="small", bufs=2))
        exps_p = ctx.enter_context(tc.tile_pool(name="exps", bufs=3))
        cstream = ctx.enter_context(tc.tile_pool(name="cstream", bufs=2))
        w2s_p = ctx.enter_context(tc.tile_pool(name="w2s", bufs=2))
        dout_p = ctx.enter_context(tc.tile_pool(name="dout", bufs=4))

        ident = consts.tile([P, P], BF)
        make_identity(nc, ident)
        ones_r128 = consts.tile([1, P], BF)
        nc.vector.memset(ones_r128, 1.0)
        ones_r64 = consts.tile([1, 64], BF)
        nc.vector.memset(ones_r64, 1.0)
        eps_t = consts.tile([P, 1], F32)
        nc.vector.memset(eps_t, EPS)
        sel_sb = consts.tile([H, H, 64], BF)
        nc.sync.dma_start(out=sel_sb, in_=sel_d.ap().rearrange("h (g d) -> h g d", d=64))

        # resident inputs
        xnT = resid.tile([P, 8, N], BF)           # [in-feat%128, in-feat//128, token]
        for f in range(8):
            eng = (nc.sync, nc.gpsimd, nc.scalar)[f % 3]
            eng.dma_start(out=xnT[:, f, :], in_=xnT_d[f * P:(f + 1) * P, :])
        qb_sb = consts.tile([P, 8], F32)
        nc.sync.dma_start(out=qb_sb, in_=qb_d.ap().rearrange("(f p) -> p f", p=P))
        mb_sb = consts.tile([P, 32], F32)
        nc.sync.dma_start(out=mb_sb, in_=mb_d.ap().rearrange("(f p) -> p f", p=P))
        kvb_sb = consts.tile([1, 2 * D], BF)
        nc.sync.dma_start(out=kvb_sb, in_=kvb_d[:])
        cbp_sb = consts.tile([1, CON_OUT], BF)
        nc.sync.dma_start(out=cbp_sb, in_=cbp_d[:])

        # resident intermediates
        qT = resid.tile([P, 8, 512], BF)
        mlpT = resid.tile([P, 32, 512], BF)
        kT = resid.tile([P, 8, N], BF)
        var_k = resid.tile([P, 8], F32)
        var_v = resid.tile([P, 8], F32)
        rstd_v = resid.tile([P, 8], F32)
        v_sb = resid.tile([P, 8, H, 65], BF)      # [k-token%128, k-tile, head, 64+ones]
        rstd_k = resid.tile([P, 8], F32)
        attnT = resid.tile([P, 8, 512], BF)

        # ---------------- expand phase 1 (q + k/v) psum scope
        exp_scope = ExitStack()
        ps_e = exp_scope.enter_context(tc.tile_pool(name="ps_e", bufs=2, space="PSUM"))
        ps_s = exp_scope.enter_context(tc.tile_pool(name="ps_s", bufs=2, space="PSUM"))

        def expand_group(og, pool):
            # out-feat group of 8 tiles; 8 resident [128,1024] weight tiles
            # (one per K-subtile, 2KB DMA lines) shared by the 8 psum groups
            wg = []
            for ks in q_active_ks:
                w = wstream.tile([P, 8 * P], BF, tag="we_t")
                nc.sync.dma_start(out=w, in_=weT_d[ks * P:(ks + 1) * P,
                                                   og * 8 * P:(og + 1) * 8 * P])
                wg.append((ks, w))
            for oi in range(8):
                of = og * 8 + oi
                ps = pool.tile([P, 512], F32, tag="ps_x")
                for ks, w in wg:
                    s0 = ks_start[ks]
                    nc.tensor.matmul(ps[:, s0:], w[:, oi * P:(oi + 1) * P],
                                     xnT[:, ks, s0:512],
                                     start=(ks == 0), stop=(ks == q_last_ks))
                if of < 8:
                    nc.vector.tensor_scalar_add(out=qT[:, of, :], in0=ps,
                                                scalar1=qb_sb[:, of:of + 1])
                else:
                    f = of - 24
                    nc.vector.tensor_scalar_add(out=mlpT[:, f, :], in0=ps,
                                                scalar1=mb_sb[:, f:f + 1])

        expand_group(0, ps_e)                          # q

        # ---------------- expand: k,v (token-major) + LN2 + k transpose
        # two sequential halves (k then v) so only half the kv weights are
        # resident at a time
        for which in ('k', 'v'):
            coff = D if which == 'k' else 2 * D
            kvw = kvraw_p.tile([P, 8, D], BF, tag="kvw", bufs=1)
            for ks in range(8):
                eng = (nc.gpsimd, nc.sync)[ks % 2]
                eng.dma_start(out=kvw[:, ks, :],
                              in_=weT_d[ks * P:(ks + 1) * P, coff:coff + D])
            for tt in range(8):
                raw = kvraw_p.tile([P, D], F32, tag="raw", bufs=2)
                nks = kv_nks[tt]
                for cf in range(2):               # 512-col tiles
                    ps = ps_e.tile([P, 512], F32, tag="ps_e")
                    nc.tensor.matmul(
                        ps, ones_r128,
                        kvb_sb[0:1, coff - D + cf * 512:coff - D + (cf + 1) * 512],
                        start=True, stop=False)
                    for ks in range(nks):
                        nc.tensor.matmul(ps, xnT[:, ks, tt * P:(tt + 1) * P],
                                         kvw[:, ks, cf * 512:(cf + 1) * 512],
                                         start=False, stop=(ks == nks - 1))
                    nc.vector.tensor_copy(out=raw[:, cf * 512:(cf + 1) * 512], in_=ps)

                stats = stat_p.tile([P, 2, 6], F32, tag="bn_st")
                for i in range(2):
                    nc.vector.bn_stats(out=stats[:, i, :], in_=raw[:, i * 512:(i + 1) * 512])
                mv = stat_p.tile([P, 2], F32, tag="bn_mv")
                nc.vector.bn_aggr(out=mv, in_=stats)
                if which == 'k':
                    nc.vector.tensor_copy(out=var_k[:, tt:tt + 1], in_=mv[:, 1:2])
                    kc = kvraw_p.tile([P, D], BF, tag="kc", bufs=2)
                    nc.vector.tensor_scalar_sub(out=kc, in0=raw, scalar1=mv[:, 0:1])
                    for fg in range(2):           # 4 PE transposes per psum bank
                        tp = ps_s.tile([P, 512], BF, tag="scr")
                        for j in range(4):
                            f = fg * 4 + j
                            nc.tensor.transpose(tp[:, j * P:(j + 1) * P],
                                                kc[:, f * P:(f + 1) * P], ident)
                        nc.vector.tensor_copy(
                            out=kT[:, fg * 4:(fg + 1) * 4, tt * P:(tt + 1) * P], in_=tp)
                else:
                    nc.vector.tensor_copy(out=var_v[:, tt:tt + 1], in_=mv[:, 1:2])
                    nc.vector.tensor_scalar_sub(
                        out=v_sb[:, tt, :, 0:64],
                        in0=raw.rearrange("p (h d) -> p h d", h=H),
                        scalar1=mv[:, 0:1])
                    nc.vector.memset(v_sb[:, tt, :, 64:65], 1.0)
        # batched rstd: one Ln + one Exp per tensor (avoids ACT table thrash)
        for var, rstd in ((var_k, rstd_k), (var_v, rstd_v)):
            lnv = stat_p.tile([P, 8], F32, tag="lnv8")
            nc.scalar.activation(out=lnv, in_=var, func=AF.Ln, bias=eps_t, scale=1.0)
            nc.scalar.activation(out=rstd, in_=lnv, func=AF.Exp, bias=0.0, scale=-0.5)
        for tt in range(8):
            nc.vector.tensor_scalar_mul(out=v_sb[:, tt, :, 0:64],
                                        in0=v_sb[:, tt, :, 0:64],
                                        scalar1=rstd_v[:, tt:tt + 1])

        exp_scope.close()
        # ---------------- attention + mlp expand (attention is ACT-exp-bound;
        # the mlp expand GEMM overlaps it on the PE)
        attnU = resid.tile([P, 8, 512], BF)       # unnormalized head pairs
        den = resid.tile([H, 512], F32)
        att_scope = ExitStack()
        ps_sp = att_scope.enter_context(tc.tile_pool(name="ps_sp", bufs=2, space="PSUM"))
        ps_a = att_scope.enter_context(tc.tile_pool(name="ps_a", bufs=2, space="PSUM"))
        ps_m = att_scope.enter_context(tc.tile_pool(name="ps_m", bufs=2, space="PSUM"))
        for hf in range(8):
            if hf % 2 == 0 and hf // 2 < 4:
                expand_group(3 + hf // 2, ps_m)        # mlp groups between pairs
            h0, h1 = 2 * hf, 2 * hf + 1
            aps0 = ps_a.tile([65, 512], F32, tag="aps")
            aps1 = ps_a.tile([65, 512], F32, tag="aps")
            for kt in range(8):
                sps = ps_sp.tile([P, 1024], F32, tag="sps")
                nc.tensor.matmul(sps[:, 0:512], kT[0:64, hf, kt * P:(kt + 1) * P],
                                 qT[0:64, hf, :], start=True, stop=True)
                nc.tensor.matmul(sps[:, 512:1024], kT[64:128, hf, kt * P:(kt + 1) * P],
                                 qT[64:128, hf, :], start=True, stop=True)
                es = exps_p.tile([P, 1024], BF, tag="es")
                nc.scalar.activation(out=es, in_=sps, func=AF.Exp,
                                     bias=0.0, scale=rstd_k[:, kt:kt + 1])
                nc.tensor.matmul(aps0, v_sb[:, kt, h0, :], es[:, 0:512],
                                 start=(kt == 0), stop=(kt == 7))
                nc.tensor.matmul(aps1, v_sb[:, kt, h1, :], es[:, 512:1024],
                                 start=(kt == 0), stop=(kt == 7))
            nc.vector.tensor_copy(out=attnU[0:64, hf, :], in_=aps0[0:64, :])
            nc.vector.tensor_copy(out=attnU[64:128, hf, :], in_=aps1[0:64, :])
            for h, aps in ((h0, aps0), (h1, aps1)):
                dtmp = small_p.tile([1, 512], F32, tag="dtmp")
                nc.vector.tensor_copy(out=dtmp, in_=aps[64:65, :])
                nc.sync.dma_start(out=den[h:h + 1, :], in_=dtmp)
        att_scope.close()
        # gelu burst: two giant in-place ops (bounds ACT table switches at 2).
        # zb is data-dependent on the last attention pair so the slabs cannot
        # preempt the exp stream mid-attention.
        zb = small_p.tile([P, 1], F32, tag="zb", bufs=1)
        nc.vector.tensor_scalar_mul(out=zb, in0=attnU[:, 7, 0:1], scalar1=0.0)
        for hb in range(2):
            nc.scalar.activation(out=mlpT[:, hb * 16:(hb + 1) * 16, :],
                                 in_=mlpT[:, hb * 16:(hb + 1) * 16, :],
                                 func=AF.Gelu, bias=zb, scale=1.0)
        # normalization tail (overlaps contract: 2 psum banks)
        tail_scope = ExitStack()
        ps_b = tail_scope.enter_context(tc.tile_pool(name="ps_b", bufs=2, space="PSUM"))
        recs = small_p.tile([H, 512], BF, tag="recs", bufs=1)
        with nc.allow_low_precision(reason="softmax denom recip in bf16"):
            nc.vector.reciprocal(out=recs, in_=den)
        for h in range(H):
            hf, hp = h // 2, (h % 2) * 64
            bps = ps_b.tile([64, 512], F32, tag="bps")
            nc.tensor.matmul(bps, sel_sb[:, h, :], recs, start=True, stop=True)
            recb = small_p.tile([P, 512], BF, tag="recb")
            nc.vector.tensor_copy(out=recb[hp:hp + 64, :], in_=bps)
            nc.vector.tensor_mul(out=attnT[hp:hp + 64, hf, :],
                                 in0=attnU[hp:hp + 64, hf, :],
                                 in1=recb[hp:hp + 64, :])

        att_scope.close()
        # ---------------- contract + combine (scoped psum: 5 banks)
        con_scope = ExitStack()
        ps_c = con_scope.enter_context(tc.tile_pool(name="ps_c", bufs=6, space="PSUM"))
        # 4 uniform 512-col passes over the permuted output space; each weight
        # tile [128,512] is loaded once and shared by all live token tiles.
        # P01 = levels 0+1 (pair-combine inside the pass), P2 = level 2,
        # P3a/P3b = the two halves of level 3 (combined at the end).
        passes = [
            dict(p0=0, tstart=min(ct_tstart[0], ct_tstart[1]), kind='p01'),
            dict(p0=512, tstart=ct_tstart[2], kind='p2'),
            dict(p0=1024, tstart=ct_tstart[3], kind='p3a'),
            dict(p0=1536, tstart=ct_tstart[3], kind='p3b'),
        ]
        t1_3a = {}
        for pz in passes:
            p0, t_lo, kind = pz['p0'], pz['tstart'], pz['kind']
            tts = list(range(t_lo, 4))
            cpss = {}
            for t in tts:
                cps = ps_c.tile([P, 512], F32, tag="cps")
                nc.tensor.matmul(cps, ones_r128, cbp_sb[0:1, p0:p0 + 512],
                                 start=True, stop=False)
                cpss[t] = cps
            pz = p0 // 512
            for qi, q4 in enumerate(list(range(2, 10)) + [0, 1]):   # mlp quads first
                wct = cstream.tile([P, 4, 512], BF, tag="wct")
                nc.gpsimd.dma_start(out=wct[:, 0:2, :], in_=wcT_d[q4, pz, :, 0:2, :])
                nc.sync.dma_start(out=wct[:, 2:4, :], in_=wcT_d[q4, pz, :, 2:4, :])
                for b4 in range(4):
                    ks = q4 * 4 + b4
                    if ks < 8:
                        lhsT = attnT[:, ks, t * P:(t + 1) * P]
                    else:
                        lhsT = mlpT[:, ks - 8, t * P:(t + 1) * P]
                    for t in tts:
                        if ks < 8:
                            lhsT = attnT[:, ks, t * P:(t + 1) * P]
                        else:
                            lhsT = mlpT[:, ks - 8, t * P:(t + 1) * P]
                        nc.tensor.matmul(cpss[t], lhsT, wct[:, b4, :], start=False,
                                         stop=(qi == 9 and b4 == 3))
            for t in tts:
                w2t = w2s_p.tile([P, 512], F32, tag="w2t")
                nc.gpsimd.dma_start(out=w2t, in_=w2_d[t * P:(t + 1) * P, p0:p0 + 512])
                t1 = dout_p.tile([P, 512], F32,
                                 tag=("t1keep" if kind == 'p3a' else "t1"),
                                 bufs=(4 if kind == 'p3a' else 2))
                nc.vector.tensor_mul(out=t1, in0=cpss[t], in1=w2t)
                if kind == 'p01':
                    for L in (0, 1):
                        if t < ct_tstart[L]:
                            continue
                        o0, ow = LEVELS[L]
                        q0 = PERM_0[L] - p0
                        dt_ = dout_p.tile([P, 128], F32, tag="dt", bufs=2)
                        nc.vector.tensor_add(out=dt_, in0=t1[:, q0:q0 + ow],
                                             in1=t1[:, q0 + ow:q0 + 2 * ow])
                        nc.sync.dma_start(out=delta_d[t * P:(t + 1) * P, o0:o0 + ow],
                                          in_=dt_)
                elif kind == 'p2':
                    o0, ow = LEVELS[2]
                    dt_ = dout_p.tile([P, 256], F32, tag="dt2", bufs=2)
                    nc.vector.tensor_add(out=dt_, in0=t1[:, 0:ow], in1=t1[:, ow:2 * ow])
                    nc.sync.dma_start(out=delta_d[t * P:(t + 1) * P, o0:o0 + ow], in_=dt_)
                elif kind == 'p3a':
                    t1_3a[t] = t1
                else:
                    o0, ow = LEVELS[3]
                    dt_ = dout_p.tile([P, 512], F32, tag="dt3", bufs=2)
                    nc.vector.tensor_add(out=dt_, in0=t1_3a[t], in1=t1)
                    nc.sync.dma_start(out=delta_d[t * P:(t + 1) * P, o0:o0 + ow], in_=dt_)

        con_scope.close()
        tail_scope.close()
        # ---------------- debug dumps
        if 'd_qT' in dbg:
            for f in range(8):
                nc.sync.dma_start(out=dbg['d_qT'][f * P:(f + 1) * P, :], in_=qT[:, f, :])
        if 'd_mlpT' in dbg:
            for f in range(32):
                nc.sync.dma_start(out=dbg['d_mlpT'][f * P:(f + 1) * P, :], in_=mlpT[:, f, :])
        if 'd_kT' in dbg:
            for f in range(8):
                nc.sync.dma_start(out=dbg['d_kT'][f * P:(f + 1) * P, :], in_=kT[:, f, :])
        if 'd_vsb' in dbg:
            for tt in range(8):
                nc.sync.dma_start(out=dbg['d_vsb'][tt * P:(tt + 1) * P, :],
                                  in_=v_sb[:, tt, :, :])
        if 'd_rstd' in dbg:
            nc.sync.dma_start(out=dbg['d_rstd'].ap().rearrange("(t p) -> p t", p=P),
                              in_=rstd_k)
        if 'd_attnT' in dbg:
            for f in range(8):
                nc.sync.dma_start(out=dbg['d_attnT'][f * P:(f + 1) * P, :], in_=attnT[:, f, :])

    nc.compile()
    return nc


# ---------------------------------------------------------------- entrypoint
_CACHE = {}

def _get_program(cfg_key, cfg):
    if cfg_key not in _CACHE:
        _CACHE[cfg_key] = build_program(cfg)
    return _CACHE[cfg_key]


def kernel(**inputs):
    """Full-input NestedParallelBlock forward on 8 NeuronCores."""
    shared, cores, cfg, aux = host_prep(inputs, nested=True)
    cfg_key = (tuple(cfg['ks_start']), tuple(cfg['kv_nks']), tuple(cfg['ct_tstart']))
    nc = _get_program(cfg_key, cfg)
    in_maps = []
    for c in range(8):
        in_maps.append(dict(xnT=cores[c]['xnT'], weT=shared['weT'], qb=shared['qb'],
                            kvb=shared['kvb'], mb=shared['mb'], wcT=shared['wcT'],
                            cbp=shared['cbp'], w2=cores[c]['w2'], sel=shared['sel']))
    trace = os.environ.get('BASS_KERNEL_TRACE', '') == '1'
    res = run_bass_kernel_spmd(nc, in_maps, list(range(8)), trace=trace)
    LAST_RESULT['exec_time_ns'] = res.exec_time_ns
    x = np.asarray(inputs['x'], np.float32)
    out = x.copy()
    for c in range(8):
        b, own = cores[c]['batch'], cores[c]['own']
        out[b, own, :] += res.results[c]['delta']
    return out


LAST_RESULT = {}
